# revision 14
# baseline (speedup 1.0000x reference)
"""Deformable Conv v2 (DCNv2) Trainium2 Bass kernel.

Problem: x[4,256,64,64], w_off[27,256,3,3], b_off[27], w_conv[256,256,3,3]
  -> out[4,256,64,64]  (offset conv + bilinear sampling + 9-point GEMM)

Sharding: 8 cores = 4 batches x 2 H-halves. Each core computes out for its
(batch, 32-row half): 2048 output pixels.

Per-core pipeline (single SPMD program):
  1. offset conv as 9 shifted-tap GEMMs on a C-major padded bf16 slice
  2. PE-transpose om to pixel-major, compute bilinear coefs (fp32) + indices
  3. dma_gather of overlapping [2C] rows (x-corner pairs) from the padded
     channels-last bf16 image in DRAM
  4. per (k, pixel-tile): 4 tensor-scalar corner mults (bf16) + paired add;
     y-corner sum folded into PE transpose with PSUM accumulation
  5. main GEMM: out[o,p] += W_k[c,o].T @ val_k[c,p], 18 K-tiles, bf16
"""

import numpy as np
import ml_dtypes

import concourse.bacc as bacc
import concourse.bass as bass
import concourse.mybir as mybir
import concourse.tile as tile
from concourse.bass_utils import run_bass_kernel_spmd

F32 = mybir.dt.float32
BF16 = mybir.dt.bfloat16
I16 = mybir.dt.int16
OP = mybir.AluOpType
AF = mybir.ActivationFunctionType

B, C, H, W, O, K = 4, 256, 64, 64, 256, 9
PADR = 2                      # zero-pad ring width
Hp, Wp = H + 2 * PADR, W + 2 * PADR            # 68, 68
NPIX = 2048                   # output pixels per core (32 rows x 64)
NT = NPIX // 128              # 16 pixel tiles
OMW = 34 * Wp                 # om computed on full 68-wide rows: 2312? (see below)
N_CORES = 8

Bb = ml_dtypes.bfloat16

# om is computed for 32 output rows on full 68-wide (incl pad) columns
OMCOLS = 32 * Wp              # 2176
XCM_COLS = 36 * Wp            # 2448 (om conv input slice: rows h0..h0+35)
OM_BLOCKS = [(0, 512), (512, 512), (1024, 512), (1536, 512), (2048, 128)]


STAGE = 5


def build_program():
    nc = bacc.Bacc("TRN2", target_bir_lowering=False, debug=False,
                   num_devices=N_CORES)
    xcl_d = nc.dram_tensor("xcl", [Hp * Wp * C], BF16, kind="ExternalInput")
    xcm_d = nc.dram_tensor("xcm", [2, 128, XCM_COLS], BF16, kind="ExternalInput")
    woff_d = nc.dram_tensor("woff", [2, 128, 9 * 27], BF16, kind="ExternalInput")
    wcv_d = nc.dram_tensor("wcv", [2, 128, 2 * 9 * 128], BF16, kind="ExternalInput")
    gyk_d = nc.dram_tensor("gyk", [128, 144], F32, kind="ExternalInput")
    gxk_d = nc.dram_tensor("gxk", [128, 144], F32, kind="ExternalInput")
    boff_d = nc.dram_tensor("boff", [27, 1], F32, kind="ExternalInput")
    identb_d = nc.dram_tensor("identb", [128, 128], BF16, kind="ExternalInput")
    identf_d = nc.dram_tensor("identf", [32, 32], F32, kind="ExternalInput")
    srep_d = nc.dram_tensor("srep", [128, 8 * 128], F32, kind="ExternalInput")
    y_d = nc.dram_tensor("y", [2, 128, NPIX], F32, kind="ExternalOutput")

    with tile.TileContext(nc) as tc:
        _emit(nc, tc, xcl_d, xcm_d, woff_d, wcv_d, gyk_d, gxk_d, boff_d,
              identb_d, identf_d, srep_d, y_d)
    nc.compile()
    return nc


def _emit(nc, tc, xcl_d, xcm_d, woff_d, wcv_d, gyk_d, gxk_d, boff_d,
          identb_d, identf_d, srep_d, y_d):
    with tc.tile_pool(name="const", bufs=1) as cpool:
        _emit_body(nc, tc, cpool, xcl_d, xcm_d, woff_d, wcv_d, gyk_d, gxk_d,
                   boff_d, identb_d, identf_d, srep_d, y_d)


def _finish_stub(nc, tc, pool, y_d):
    z = pool.tile([128, NPIX], F32, tag="zstub", name="zstub")
    nc.vector.memset(z[:, :], 0.0)
    nc.sync.dma_start(y_d.ap()[0], z[:, :])
    nc.sync.dma_start(y_d.ap()[1], z[:, :])


def _emit_body(nc, tc, cpool, xcl_d, xcm_d, woff_d, wcv_d, gyk_d, gxk_d,
               boff_d, identb_d, identf_d, srep_d, y_d):
    # --- persistent constants ---
    wcv = [cpool.tile([128, 2 * 9 * 128], BF16, tag=f"wcv{ct}", name=f"wcv{ct}") for ct in range(2)]
    for ct in range(2):
        nc.sync.dma_start(wcv[ct][:, :], wcv_d.ap()[ct])
    woff = [cpool.tile([128, 9 * 27], BF16, tag=f"woff{ct}", name=f"woff{ct}") for ct in range(2)]
    for ct in range(2):
        nc.sync.dma_start(woff[ct][:, :], woff_d.ap()[ct])
    gyk = cpool.tile([128, 144], F32, tag="gyk", name="gyk")
    nc.sync.dma_start(gyk[:, :], gyk_d.ap()[:, :])
    gxk = cpool.tile([128, 144], F32, tag="gxk", name="gxk")
    nc.sync.dma_start(gxk[:, :], gxk_d.ap()[:, :])
    boff = cpool.tile([27, 1], F32, tag="boff", name="boff")
    nc.sync.dma_start(boff[:, :], boff_d.ap()[:, :])
    identb = cpool.tile([128, 128], BF16, tag="identb", name="identb")
    nc.sync.dma_start(identb[:, :], identb_d.ap()[:, :])
    identf = cpool.tile([32, 32], F32, tag="identf", name="identf")
    nc.sync.dma_start(identf[:, :], identf_d.ap()[:, :])
    srep = cpool.tile([128, 8 * 128], F32, tag="srep", name="srep")
    nc.sync.dma_start(srep[:, :], srep_d.ap()[:, :])

    # persistent: corner coefs, wrapped indices
    c00 = cpool.tile([128, 144], F32, tag="c00", name="c00")
    c01 = cpool.tile([128, 144], F32, tag="c01", name="c01")
    c10 = cpool.tile([128, 144], F32, tag="c10", name="c10")
    c11 = cpool.tile([128, 144], F32, tag="c11", name="c11")
    # bf16 copies: all-bf16 operands keep the DVE in 2x perf mode
    cb = [cpool.tile([128, 144], BF16, tag=f"cb{i}", name=f"cb{i}")
          for i in range(4)]
    iw = cpool.tile([128, 18 * 128], I16, tag="iw", name="iw")

    # ---------------- Phase A: offset conv + coefs (scoped pools) ---------
    with tc.tile_pool(name="early", bufs=1) as epool, \
         tc.tile_pool(name="om_ps", bufs=2, space="PSUM") as om_ps, \
         tc.tile_pool(name="idx_ps", bufs=4, space="PSUM") as idx_ps, \
         tc.tile_pool(name="omp_ps", bufs=1, space="PSUM") as omp_ps:
        xcm = [epool.tile([128, XCM_COLS], BF16, tag=f"xcm{ct}", name=f"xcm{ct}") for ct in range(2)]
        for ct in range(2):
            nc.sync.dma_start(xcm[ct][:, :], xcm_d.ap()[ct])

        if STAGE < 1:
            _finish_stub(nc, tc, cpool, y_d)
            return
        om_s = epool.tile([27, OMCOLS], F32, tag="om_s", name="om_s")
        for nboff, nbsz in OM_BLOCKS:
            omp = om_ps.tile([27, 512], F32, tag="omps", name="omps")
            first = True
            for tap in range(9):
                ky, kx = tap // 3, tap % 3
                toff = (ky + 1) * Wp + kx - 1
                for ct in range(2):
                    nc.tensor.matmul(
                        omp[:, 0:nbsz],
                        woff[ct][:, tap * 27:(tap + 1) * 27],
                        xcm[ct][:, toff + nboff: toff + nboff + nbsz],
                        start=first, stop=(tap == 8 and ct == 1),
                    )
                    first = False
            nc.scalar.activation(om_s[:, nboff:nboff + nbsz], omp[:, 0:nbsz],
                                 AF.Identity, bias=boff[:, 0:1])

        if STAGE < 2:
            _finish_stub(nc, tc, cpool, y_d)
            return
        # om -> pixel-major via PE transpose (compact valid pixels first:
        # matmul operands must have a single free dim)
        om_v = epool.tile([27, NPIX], F32, tag="om_v", name="om_v")
        om_h = om_s[:, :]
        nc.vector.tensor_copy(
            om_v[:, :],
            bass.AP(om_h.tensor, om_h.offset + 2,
                    [list(om_h.ap[0]), [Wp, 32], [1, 64]]))
        omp_pm = omp_ps.tile([128, NT * 27], F32, tag="omppm", name="omppm")
        for t in range(NT):
            nc.tensor.matmul(omp_pm[:, 27 * t:27 * (t + 1)],
                             om_v[:, 128 * t:128 * (t + 1)],
                             identf[0:27, 0:27], is_transpose=True,
                             start=True, stop=True)
        omp_s = epool.tile([128, NT * 27], F32, tag="omp_s", name="omp_s")
        nc.scalar.copy(omp_s[:, :], omp_pm[:, :])

        # --- coef pipeline (pixel-major [128, 16, 9] strided views) ---
        base = omp_s[:, :]
        p0 = list(base.ap[0])

        def omview(ch_off, ch_step):
            return bass.AP(base.tensor, base.offset + ch_off,
                           [p0, [27, NT], [ch_step, 9]])

        def wtile(tag):
            return epool.tile([128, 144], F32, tag=tag, name=tag)

        py = wtile("py")
        nc.vector.tensor_tensor(py[:, :], omview(0, 2), gyk[:, :], OP.add)
        px = wtile("px")
        nc.vector.tensor_tensor(px[:, :], omview(1, 2), gxk[:, :], OP.add)

        # floor via +16-bias cast roundtrip (correct for trunc OR round-to-
        # nearest casts; bias keeps the operand positive, clamp absorbs it).
        I32 = mybir.dt.int32
        BIAS = 16.0

        def floor_frac(p, pre):
            pt = wtile(pre + "t")
            nc.vector.tensor_scalar(pt[:, :], p[:, :], BIAS, None, OP.add)
            pi = epool.tile([128, 144], I32, tag=pre + "i", name=pre + "i")
            nc.vector.tensor_copy(pi[:, :], pt[:, :])
            pf = wtile(pre + "f")
            nc.vector.tensor_copy(pf[:, :], pi[:, :])
            gg = wtile(pre + "g")
            nc.vector.tensor_tensor(gg[:, :], pf[:, :], pt[:, :], OP.is_gt)
            fb = wtile(pre + "fb")   # floor(p)+BIAS
            nc.vector.tensor_tensor(fb[:, :], pf[:, :], gg[:, :], OP.subtract)
            fr = wtile(pre + "fr")   # frac(p)
            nc.vector.tensor_tensor(fr[:, :], pt[:, :], fb[:, :], OP.subtract)
            return fb, fr

        y0b, wy = floor_frac(py, "y")
        x0b, wx = floor_frac(px, "x")
        # clamp (still biased by +16): [-2, H] -> [14, H+16]
        nc.vector.tensor_scalar(y0b[:, :], y0b[:, :], 14.0, float(H) + BIAS,
                                OP.max, OP.min)
        nc.vector.tensor_scalar(x0b[:, :], x0b[:, :], 14.0, float(W) + BIAS,
                                OP.max, OP.min)
        # idx = 68*(y0+2) + x0+2 = 68*y0b + x0b - 966
        idxf = epool.tile([128, 2, 144], F32, tag="idxf", name="idxf")
        nc.vector.tensor_scalar(idxf[:, 0, :], y0b[:, :], float(Wp), -966.0,
                                OP.mult, OP.add)
        nc.vector.tensor_tensor(idxf[:, 0, :], idxf[:, 0, :], x0b[:, :], OP.add)
        nc.vector.tensor_scalar(idxf[:, 1, :], idxf[:, 0, :], float(Wp), None,
                                OP.add)

        msk = wtile("msk")
        nc.scalar.activation(msk[:, :], omview(18, 1), AF.Sigmoid)
        b1 = wtile("b1")
        nc.vector.tensor_tensor(b1[:, :], wy[:, :], msk[:, :], OP.mult)
        b0 = wtile("b0")
        nc.vector.tensor_tensor(b0[:, :], msk[:, :], b1[:, :], OP.subtract)
        nc.vector.tensor_tensor(c01[:, :], b0[:, :], wx[:, :], OP.mult)
        nc.vector.tensor_tensor(c00[:, :], b0[:, :], c01[:, :], OP.subtract)
        nc.vector.tensor_tensor(c11[:, :], b1[:, :], wx[:, :], OP.mult)
        nc.vector.tensor_tensor(c10[:, :], b1[:, :], c11[:, :], OP.subtract)
        for src, dst in zip((c00, c01, c10, c11), cb):
            nc.vector.tensor_copy(dst[:, :], src[:, :])

        # --- index wrap to [16, n/16] layout via PE shuffle ---
        # Target: iw[16a + r%16, 128*ks + 8t + r//16] = idx(p=128t+r, ks)
        # for all a in [0,8) (each GPSIMD Q7 core reads its own
        # 16-partition block). Done with 8 fp32 matmuls against
        # host-built selection matrices S_j[p, m] = (p == 16j + m%16):
        # out_j[m, n] = idxv[16j + m%16, n] — the 128->16 partition wrap
        # with the 8x replication built in.
        idxv = epool.tile([128, 288], F32, tag="idxv", name="idxv")
        for y in range(2):
            nc.vector.tensor_copy(
                idxv[:, 144 * y:144 * (y + 1)].rearrange(
                    "p (k t) -> p k t", t=NT),
                idxf[:, y, :].rearrange("p (t k) -> p k t", k=9))
        iw_b = iw[:, :]
        for j in range(8):
            mmj = idx_ps.tile([128, 288], F32, tag="idxmm", name="idxmm")
            nc.tensor.matmul(mmj[:, :], srep[:, 128 * j:128 * (j + 1)],
                             idxv[:, :], start=True, stop=True)
            nc.vector.tensor_copy(
                bass.AP(iw_b.tensor, iw_b.offset + j,
                        [list(iw_b.ap[0]), [128, 18], [8, 16]]),
                mmj[:, :].rearrange("p (ks t) -> p ks t", t=NT))

    if STAGE < 3:
        _finish_stub(nc, tc, cpool, y_d)
        return
    # ---------------- Phase B: gather / apply / transpose / GEMM ----------
    xcl_h = xcl_d  # flat [Hp*Wp*C]
    win0 = bass.AP(xcl_h, 0, [[C, Hp * Wp - 1], [1, 2 * C]])

    with tc.tile_pool(name="val", bufs=9) as vpool, \
         tc.tile_pool(name="g", bufs=2) as gpool, \
         tc.tile_pool(name="ab", bufs=4) as apool, \
         tc.tile_pool(name="outs", bufs=2) as opool, \
         tc.tile_pool(name="gemm_ps", bufs=4, space="PSUM") as gemm_ps, \
         tc.tile_pool(name="tp_ps", bufs=4, space="PSUM") as tp_ps:

        vals = []
        gps0 = [gemm_ps.tile([128, 512], F32, tag="gps", name="gps") for _ in range(4)]

        def emit_gemm_k(gps, k, ot):
            for ct in range(2):
                for nb in range(4):
                    nc.tensor.matmul(
                        gps[nb][:, :],
                        wcv[ct][:, (ot * 9 + k) * 128:(ot * 9 + k + 1) * 128],
                        vals[k][:, ct, nb * 512:(nb + 1) * 512],
                        start=(k == 0 and ct == 0),
                        stop=(k == 8 and ct == 1),
                    )

        for k in range(9):
            g0 = gpool.tile([128, NT, 2 * C], BF16, tag="g0", name="g0")
            g1 = gpool.tile([128, NT, 2 * C], BF16, tag="g1", name="g1")
            # NI-idx gathers per (k, y-corner); position i = pixel i,
            # its index at iw[i%16 (+16a), 128*ks + i//16]
            NI = 2048
            NQ = NPIX // NI
            for q in range(NQ):
                nc.gpsimd.dma_gather(
                    out_ap=g0[:, (NI // 128) * q:(NI // 128) * (q + 1), :],
                    in_ap=win0,
                    idxs_ap=iw[:, 128 * k + (NI // 16) * q:
                               128 * k + (NI // 16) * (q + 1)],
                    num_idxs=NI, num_idxs_reg=NI,
                    elem_size=2 * C, elem_step=C, single_packet=False)
                nc.gpsimd.dma_gather(
                    out_ap=g1[:, (NI // 128) * q:(NI // 128) * (q + 1), :],
                    in_ap=win0,
                    idxs_ap=iw[:, 128 * (9 + k) + (NI // 16) * q:
                               128 * (9 + k) + (NI // 16) * (q + 1)],
                    num_idxs=NI, num_idxs_reg=NI,
                    elem_size=2 * C, elem_step=C, single_packet=False)

            if STAGE < 4:
                continue
            val = vpool.tile([128, 2, NPIX], BF16, tag="val", name="val")
            vals.append(val)
            for half in range(4):      # 4 pixel-quads of 4 tiles each
                tp = [tp_ps.tile([128, 512], BF16, tag="tp", name="tp") for _ in range(2)]
                for t in range(4 * half, 4 * half + 4):
                    col = t * 9 + k
                    # val_t = c00*g0lo + c10*g1lo + c01*g0hi + c11*g1hi
                    # via 1 scalar-engine mult + 3 fused DVE mult-adds
                    mb = apool.tile([128, C], BF16, tag="mb", name="mb")
                    nc.scalar.activation(mb[:, :], g0[:, t, 0:C],
                                         AF.Copy, scale=c00[:, col:col + 1])
                    m1 = apool.tile([128, C], BF16, tag="m1", name="m1")
                    nc.vector.scalar_tensor_tensor(
                        m1[:, :], g1[:, t, 0:C], cb[2][:, col:col + 1],
                        mb[:, :], OP.mult, OP.add)
                    m2 = apool.tile([128, C], BF16, tag="m2", name="m2")
                    nc.vector.scalar_tensor_tensor(
                        m2[:, :], g0[:, t, C:2 * C], cb[1][:, col:col + 1],
                        m1[:, :], OP.mult, OP.add)
                    vt = apool.tile([128, C], BF16, tag="vt", name="vt")
                    nc.vector.scalar_tensor_tensor(
                        vt[:, :], g1[:, t, C:2 * C], cb[3][:, col:col + 1],
                        m2[:, :], OP.mult, OP.add)
                    # PE transpose pixel-major val tile -> C-major (PSUM)
                    sl = slice((t % 4) * 128, (t % 4) * 128 + 128)
                    for ch in range(2):
                        nc.tensor.matmul(tp[ch][:, sl],
                                         vt[:, ch * 128:(ch + 1) * 128],
                                         identb[:, :], is_transpose=True,
                                         start=True, stop=True)
                for ch in range(2):
                    nc.scalar.copy(val[:, ch, half * 512:(half + 1) * 512],
                                   tp[ch][:, :])
            if STAGE >= 5 and k >= 1:
                emit_gemm_k(gps0, k - 1, ot=0)

        if STAGE < 5:
            _finish_stub(nc, tc, cpool, y_d)
            return
        emit_gemm_k(gps0, 8, ot=0)
        out0 = opool.tile([128, NPIX], F32, tag="outs", name="outs")
        for nb in range(4):
            nc.scalar.copy(out0[:, nb * 512:(nb + 1) * 512], gps0[nb][:, :])
        nc.sync.dma_start(y_d.ap()[0], out0[:, :])

        gps1 = [gemm_ps.tile([128, 512], F32, tag="gps", name="gps") for _ in range(4)]
        for k in range(9):
            emit_gemm_k(gps1, k, ot=1)
        out1 = opool.tile([128, NPIX], F32, tag="outs", name="outs")
        for nb in range(4):
            nc.scalar.copy(out1[:, nb * 512:(nb + 1) * 512], gps1[nb][:, :])
        nc.sync.dma_start(y_d.ap()[1], out1[:, :])


# ---------------------------------------------------------------------------
# Host side
# ---------------------------------------------------------------------------

def make_core_inputs(x, w_off, b_off, w_conv, core):
    b, s = core // 2, core % 2
    h0 = 32 * s
    xp = np.zeros((C, Hp, Wp), np.float32)
    xp[:, PADR:PADR + H, PADR:PADR + W] = x[b]
    xcl = np.ascontiguousarray(xp.transpose(1, 2, 0)).reshape(-1).astype(BbArr)
    xcm = np.ascontiguousarray(
        xp.reshape(C, Hp * Wp)[:, h0 * Wp: h0 * Wp + XCM_COLS]
    ).astype(BbArr).reshape(2, 128, XCM_COLS)

    # w_off [27, C, 3, 3] -> [2, 128, 9*27]: [ct, c, tap*27+oc]
    wof = w_off.transpose(1, 2, 3, 0).reshape(2, 128, 9, 27).reshape(2, 128, 243)
    wof = np.ascontiguousarray(wof).astype(BbArr)
    # w_conv [O, C, 3, 3] -> [2(ct), 128(c), (ot*9+k)*128+o]
    wc = w_conv.reshape(2, 128, C, 9).transpose(2, 0, 3, 1)   # [c, ot, k, o128]
    wc = np.ascontiguousarray(wc.reshape(2, 128, 2 * 9 * 128)).astype(BbArr)

    r = np.arange(128)[:, None, None]
    t = np.arange(NT)[None, :, None]
    kk = np.arange(9)[None, None, :]
    gyk = (h0 + 2 * t + r // 64 - 1 + kk // 3).astype(np.float32)
    gyk = np.broadcast_to(gyk, (128, NT, 9)).reshape(128, 144).copy()
    gxk = (r % 64 - 1 + kk % 3).astype(np.float32)
    gxk = np.broadcast_to(gxk, (128, NT, 9)).reshape(128, 144).copy()

    # selection matrices for the 128->16 index partition wrap:
    # S_j[p, m] = 1 iff p == 16j + m%16  (j = r//16 column-block)
    srep = np.zeros((128, 8, 128), np.float32)
    p = np.arange(128)
    m = np.arange(128)
    for j in range(8):
        srep[:, j, :] = (p[:, None] == 16 * j + (m[None, :] % 16))
    srep = srep.reshape(128, 1024)

    return {
        "xcl": xcl,
        "xcm": xcm,
        "woff": wof,
        "wcv": wc,
        "gyk": gyk,
        "gxk": gxk,
        "boff": b_off.reshape(27, 1).astype(np.float32),
        "identb": np.eye(128, dtype=np.float32).astype(BbArr),
        "identf": np.eye(32, dtype=np.float32),
        "srep": srep,
    }


BbArr = ml_dtypes.bfloat16

_NC = None


def kernel(x, w_off, b_off, w_conv):
    global _NC
    x = np.asarray(x, np.float32)
    w_off = np.asarray(w_off, np.float32)
    b_off = np.asarray(b_off, np.float32)
    w_conv = np.asarray(w_conv, np.float32)
    if _NC is None:
        _NC = build_program()
    in_maps = [make_core_inputs(x, w_off, b_off, w_conv, c)
               for c in range(N_CORES)]
    res = run_bass_kernel_spmd(_NC, in_maps, core_ids=list(range(N_CORES)))
    out = np.empty((B, O, H, W), np.float32)
    for c in range(N_CORES):
        b, s = c // 2, c % 2
        out[b, :, 32 * s:32 * s + 32, :] = res.results[c]["y"].reshape(O, 32, W)
    return out



# revision 30
# speedup vs baseline: 1.9817x; 1.9817x over previous
"""Deformable Conv v2 (DCNv2) Trainium2 Bass kernel.

Problem: x[4,256,64,64], w_off[27,256,3,3], b_off[27], w_conv[256,256,3,3]
  -> out[4,256,64,64]  (offset conv + bilinear sampling + 9-point GEMM)

Sharding: 8 cores = 4 batches x 2 H-halves. Each core computes out for its
(batch, 32-row half): 2048 output pixels.

Per-core pipeline (single SPMD program):
  1. offset conv as 9 shifted-tap GEMMs on a C-major padded bf16 slice
  2. PE-transpose om to pixel-major, compute bilinear coefs (fp32) + indices
  3. dma_gather of overlapping [2C] rows (x-corner pairs) from the padded
     channels-last bf16 image in DRAM
  4. per (k, pixel-tile): 4 tensor-scalar corner mults (bf16) + paired add;
     y-corner sum folded into PE transpose with PSUM accumulation
  5. main GEMM: out[o,p] += W_k[c,o].T @ val_k[c,p], 18 K-tiles, bf16
"""

import numpy as np
import ml_dtypes

import concourse.bacc as bacc
import concourse.bass as bass
import concourse.mybir as mybir
import concourse.tile as tile
from concourse.bass_utils import run_bass_kernel_spmd

F32 = mybir.dt.float32
BF16 = mybir.dt.bfloat16
I16 = mybir.dt.int16
OP = mybir.AluOpType
AF = mybir.ActivationFunctionType

B, C, H, W, O, K = 4, 256, 64, 64, 256, 9
PADR = 2                      # zero-pad ring width
Hp, Wp = H + 2 * PADR, W + 2 * PADR            # 68, 68
NPIX = 2048                   # output pixels per core (32 rows x 64)
NT = NPIX // 128              # 16 pixel tiles
OMW = 34 * Wp                 # om computed on full 68-wide rows: 2312? (see below)
N_CORES = 8

Bb = ml_dtypes.bfloat16

# om is computed for 32 output rows on full 68-wide (incl pad) columns
OMCOLS = 32 * Wp              # 2176
XCM_COLS = 36 * Wp            # 2448 (om conv input slice: rows h0..h0+35)
OM_BLOCKS = [(0, 512), (512, 512), (1024, 512), (1536, 512), (2048, 128)]


STAGE = 5
INDIRECT = True               # gather via hw-DGE indirect DMA from the 4C
                              # corner-block table (vs gpsimd dma_gather)


def build_program():
    nc = bacc.Bacc("TRN2", target_bir_lowering=False, debug=False,
                   num_devices=N_CORES)
    xcl_d = nc.dram_tensor("xcl", [Hp * Wp * C], BF16, kind="ExternalInput")
    xcl4_d = nc.dram_tensor("xcl4", [Hp * Wp, 4 * C], BF16,
                            kind="ExternalInput")
    xcm_d = nc.dram_tensor("xcm", [2, 128, XCM_COLS], BF16, kind="ExternalInput")
    woff_d = nc.dram_tensor("woff", [2, 128, 9 * 27], BF16, kind="ExternalInput")
    wcv_d = nc.dram_tensor("wcv", [2, 128, 2 * 9 * 128], BF16, kind="ExternalInput")
    gyk_d = nc.dram_tensor("gyk", [128, 144], F32, kind="ExternalInput")
    gxk_d = nc.dram_tensor("gxk", [128, 144], F32, kind="ExternalInput")
    boff_d = nc.dram_tensor("boff", [27, 1], F32, kind="ExternalInput")
    identb_d = nc.dram_tensor("identb", [128, 128], BF16, kind="ExternalInput")
    identf_d = nc.dram_tensor("identf", [32, 32], F32, kind="ExternalInput")
    srep_d = nc.dram_tensor("srep", [128, 8 * 128], F32, kind="ExternalInput")
    y_d = nc.dram_tensor("y", [2, 128, NPIX], F32, kind="ExternalOutput")

    with tile.TileContext(nc) as tc:
        _emit(nc, tc, xcl_d, xcl4_d, xcm_d, woff_d, wcv_d, gyk_d, gxk_d,
              boff_d, identb_d, identf_d, srep_d, y_d)
    nc.compile()
    return nc


def _emit(nc, tc, xcl_d, xcl4_d, xcm_d, woff_d, wcv_d, gyk_d, gxk_d,
          boff_d, identb_d, identf_d, srep_d, y_d):
    with tc.tile_pool(name="const", bufs=1) as cpool:
        _emit_body(nc, tc, cpool, xcl_d, xcl4_d, xcm_d, woff_d, wcv_d, gyk_d,
                   gxk_d, boff_d, identb_d, identf_d, srep_d, y_d)


def _finish_stub(nc, tc, pool, y_d):
    z = pool.tile([128, NPIX], F32, tag="zstub", name="zstub")
    nc.vector.memset(z[:, :], 0.0)
    nc.sync.dma_start(y_d.ap()[0], z[:, :])
    nc.sync.dma_start(y_d.ap()[1], z[:, :])


def _emit_body(nc, tc, cpool, xcl_d, xcl4_d, xcm_d, woff_d, wcv_d, gyk_d,
               gxk_d, boff_d, identb_d, identf_d, srep_d, y_d):
    # --- persistent constants ---
    wcv = [cpool.tile([128, 2 * 9 * 128], BF16, tag=f"wcv{ct}", name=f"wcv{ct}") for ct in range(2)]
    for ct in range(2):
        nc.sync.dma_start(wcv[ct][:, :], wcv_d.ap()[ct])
    woff = [cpool.tile([128, 9 * 27], BF16, tag=f"woff{ct}", name=f"woff{ct}") for ct in range(2)]
    for ct in range(2):
        nc.sync.dma_start(woff[ct][:, :], woff_d.ap()[ct])
    gyk = cpool.tile([128, 144], F32, tag="gyk", name="gyk")
    nc.sync.dma_start(gyk[:, :], gyk_d.ap()[:, :])
    gxk = cpool.tile([128, 144], F32, tag="gxk", name="gxk")
    nc.sync.dma_start(gxk[:, :], gxk_d.ap()[:, :])
    boff = cpool.tile([27, 1], F32, tag="boff", name="boff")
    nc.sync.dma_start(boff[:, :], boff_d.ap()[:, :])
    identb = cpool.tile([128, 128], BF16, tag="identb", name="identb")
    nc.sync.dma_start(identb[:, :], identb_d.ap()[:, :])
    identf = cpool.tile([32, 32], F32, tag="identf", name="identf")
    nc.sync.dma_start(identf[:, :], identf_d.ap()[:, :])
    srep = cpool.tile([128, 8 * 128], F32, tag="srep", name="srep")
    nc.sync.dma_start(srep[:, :], srep_d.ap()[:, :])

    # persistent: corner coefs, wrapped indices
    c00 = cpool.tile([128, 144], F32, tag="c00", name="c00")
    c01 = cpool.tile([128, 144], F32, tag="c01", name="c01")
    c10 = cpool.tile([128, 144], F32, tag="c10", name="c10")
    c11 = cpool.tile([128, 144], F32, tag="c11", name="c11")
    # bf16 copies: all-bf16 operands keep the DVE in 2x perf mode
    cb = [cpool.tile([128, 144], BF16, tag=f"cb{i}", name=f"cb{i}")
          for i in range(4)]
    iw = cpool.tile([128, 18 * 128], I16, tag="iw", name="iw")
    idx32 = cpool.tile([128, 9, NT], mybir.dt.int32, tag="idx32", name="idx32")

    # ---------------- Phase A: offset conv + coefs (scoped pools) ---------
    with tc.tile_pool(name="early", bufs=1) as epool, \
         tc.tile_pool(name="om_ps", bufs=2, space="PSUM") as om_ps, \
         tc.tile_pool(name="idx_ps", bufs=4, space="PSUM") as idx_ps, \
         tc.tile_pool(name="omp_ps", bufs=1, space="PSUM") as omp_ps:
        xcm = [epool.tile([128, XCM_COLS], BF16, tag=f"xcm{ct}", name=f"xcm{ct}") for ct in range(2)]
        for ct in range(2):
            nc.sync.dma_start(xcm[ct][:, :], xcm_d.ap()[ct])

        if STAGE < 1:
            _finish_stub(nc, tc, cpool, y_d)
            return
        om_s = epool.tile([27, OMCOLS], F32, tag="om_s", name="om_s")
        for nboff, nbsz in OM_BLOCKS:
            omp = om_ps.tile([27, 512], F32, tag="omps", name="omps")
            first = True
            for tap in range(9):
                ky, kx = tap // 3, tap % 3
                toff = (ky + 1) * Wp + kx - 1
                for ct in range(2):
                    nc.tensor.matmul(
                        omp[:, 0:nbsz],
                        woff[ct][:, tap * 27:(tap + 1) * 27],
                        xcm[ct][:, toff + nboff: toff + nboff + nbsz],
                        start=first, stop=(tap == 8 and ct == 1),
                    )
                    first = False
            nc.scalar.activation(om_s[:, nboff:nboff + nbsz], omp[:, 0:nbsz],
                                 AF.Identity, bias=boff[:, 0:1])

        if STAGE < 2:
            _finish_stub(nc, tc, cpool, y_d)
            return
        # om -> pixel-major via PE transpose (compact valid pixels first:
        # matmul operands must have a single free dim)
        om_v = epool.tile([27, NPIX], F32, tag="om_v", name="om_v")
        om_h = om_s[:, :]
        nc.vector.tensor_copy(
            om_v[:, :],
            bass.AP(om_h.tensor, om_h.offset + 2,
                    [list(om_h.ap[0]), [Wp, 32], [1, 64]]))
        omp_pm = omp_ps.tile([128, NT * 27], F32, tag="omppm", name="omppm")
        for t in range(NT):
            nc.tensor.matmul(omp_pm[:, 27 * t:27 * (t + 1)],
                             om_v[:, 128 * t:128 * (t + 1)],
                             identf[0:27, 0:27], is_transpose=True,
                             start=True, stop=True)
        omp_s = epool.tile([128, NT * 27], F32, tag="omp_s", name="omp_s")
        nc.scalar.copy(omp_s[:, :], omp_pm[:, :])

        # --- coef pipeline (pixel-major [128, 16, 9] strided views) ---
        base = omp_s[:, :]
        p0 = list(base.ap[0])

        def omview(ch_off, ch_step):
            return bass.AP(base.tensor, base.offset + ch_off,
                           [p0, [27, NT], [ch_step, 9]])

        def wtile(tag):
            return epool.tile([128, 144], F32, tag=tag, name=tag)

        py = wtile("py")
        nc.vector.tensor_tensor(py[:, :], omview(0, 2), gyk[:, :], OP.add)
        px = wtile("px")
        nc.vector.tensor_tensor(px[:, :], omview(1, 2), gxk[:, :], OP.add)

        # floor via +16-bias cast roundtrip (correct for trunc OR round-to-
        # nearest casts; bias keeps the operand positive, clamp absorbs it).
        I32 = mybir.dt.int32
        BIAS = 16.0

        def floor_frac(p, pre):
            pt = wtile(pre + "t")
            nc.vector.tensor_scalar(pt[:, :], p[:, :], BIAS, None, OP.add)
            pi = epool.tile([128, 144], I32, tag=pre + "i", name=pre + "i")
            nc.vector.tensor_copy(pi[:, :], pt[:, :])
            pf = wtile(pre + "f")
            nc.vector.tensor_copy(pf[:, :], pi[:, :])
            gg = wtile(pre + "g")
            nc.vector.tensor_tensor(gg[:, :], pf[:, :], pt[:, :], OP.is_gt)
            fb = wtile(pre + "fb")   # floor(p)+BIAS
            nc.vector.tensor_tensor(fb[:, :], pf[:, :], gg[:, :], OP.subtract)
            fr = wtile(pre + "fr")   # frac(p)
            nc.vector.tensor_tensor(fr[:, :], pt[:, :], fb[:, :], OP.subtract)
            return fb, fr

        y0b, wy = floor_frac(py, "y")
        x0b, wx = floor_frac(px, "x")
        # clamp (still biased by +16): [-2, H] -> [14, H+16]
        nc.vector.tensor_scalar(y0b[:, :], y0b[:, :], 14.0, float(H) + BIAS,
                                OP.max, OP.min)
        nc.vector.tensor_scalar(x0b[:, :], x0b[:, :], 14.0, float(W) + BIAS,
                                OP.max, OP.min)
        # idx = 68*(y0+2) + x0+2 = 68*y0b + x0b - 966
        idxf = epool.tile([128, 2, 144], F32, tag="idxf", name="idxf")
        nc.vector.tensor_scalar(idxf[:, 0, :], y0b[:, :], float(Wp), -966.0,
                                OP.mult, OP.add)
        nc.vector.tensor_tensor(idxf[:, 0, :], idxf[:, 0, :], x0b[:, :], OP.add)
        nc.vector.tensor_scalar(idxf[:, 1, :], idxf[:, 0, :], float(Wp), None,
                                OP.add)

        msk = wtile("msk")
        nc.scalar.activation(msk[:, :], omview(18, 1), AF.Sigmoid)
        b1 = wtile("b1")
        nc.vector.tensor_tensor(b1[:, :], wy[:, :], msk[:, :], OP.mult)
        b0 = wtile("b0")
        nc.vector.tensor_tensor(b0[:, :], msk[:, :], b1[:, :], OP.subtract)
        nc.vector.tensor_tensor(c01[:, :], b0[:, :], wx[:, :], OP.mult)
        nc.vector.tensor_tensor(c00[:, :], b0[:, :], c01[:, :], OP.subtract)
        nc.vector.tensor_tensor(c11[:, :], b1[:, :], wx[:, :], OP.mult)
        nc.vector.tensor_tensor(c10[:, :], b1[:, :], c11[:, :], OP.subtract)
        for src, dst in zip((c00, c01, c10, c11), cb):
            nc.vector.tensor_copy(dst[:, :], src[:, :])

        if INDIRECT:
            # pixel-major int32 token ids for the hw-DGE indirect gather:
            # idx32[r, k, t] = (y0+2)*Wp + (x0+2) for pixel p=128t+r, tap k
            nc.vector.tensor_copy(
                idx32[:, :, :],
                idxf[:, 0, :].rearrange("p (t k) -> p k t", k=9))
        else:
            # --- index wrap to [16, n/16] layout via PE shuffle ---
            # Target: iw[16a + r%16, 128*ks + 8t + r//16] = idx(p=128t+r, ks)
            # for all a in [0,8) (each GPSIMD Q7 core reads its own
            # 16-partition block). Done with 8 fp32 matmuls against
            # host-built selection matrices S_j[p, m] = (p == 16j + m%16):
            # out_j[m, n] = idxv[16j + m%16, n] — the 128->16 partition wrap
            # with the 8x replication built in.
            idxv = epool.tile([128, 288], F32, tag="idxv", name="idxv")
            for y in range(2):
                nc.vector.tensor_copy(
                    idxv[:, 144 * y:144 * (y + 1)].rearrange(
                        "p (k t) -> p k t", t=NT),
                    idxf[:, y, :].rearrange("p (t k) -> p k t", k=9))
            iw_b = iw[:, :]
            for j in range(8):
                mmj = idx_ps.tile([128, 288], F32, tag="idxmm", name="idxmm")
                nc.tensor.matmul(mmj[:, :], srep[:, 128 * j:128 * (j + 1)],
                                 idxv[:, :], start=True, stop=True)
                nc.vector.tensor_copy(
                    bass.AP(iw_b.tensor, iw_b.offset + j,
                            [list(iw_b.ap[0]), [128, 18], [8, 16]]),
                    mmj[:, :].rearrange("p (ks t) -> p ks t", t=NT))

    if STAGE < 3:
        _finish_stub(nc, tc, cpool, y_d)
        return
    # ---------------- Phase B: gather / apply / transpose / GEMM ----------
    xcl_h = xcl_d  # flat [Hp*Wp*C]
    win0 = bass.AP(xcl_h, 0, [[C, Hp * Wp - 1], [1, 2 * C]])

    with tc.tile_pool(name="val", bufs=9) as vpool, \
         tc.tile_pool(name="g", bufs=2) as gpool, \
         tc.tile_pool(name="ab", bufs=4) as apool, \
         tc.tile_pool(name="outs", bufs=2) as opool, \
         tc.tile_pool(name="gemm_ps", bufs=4, space="PSUM") as gemm_ps, \
         tc.tile_pool(name="tp_ps", bufs=4, space="PSUM") as tp_ps:

        vals = []
        gps0 = [gemm_ps.tile([128, 512], F32, tag="gps", name="gps") for _ in range(4)]

        def emit_gemm_k(gps, k, ot):
            for ct in range(2):
                for nb in range(4):
                    nc.tensor.matmul(
                        gps[nb][:, :],
                        wcv[ct][:, (ot * 9 + k) * 128:(ot * 9 + k + 1) * 128],
                        vals[k][:, ct, nb * 512:(nb + 1) * 512],
                        start=(k == 0 and ct == 0),
                        stop=(k == 8 and ct == 1),
                    )

        for k in range(9):
            if INDIRECT:
                # one hw-DGE indirect DMA per pixel tile: partition p of
                # out <- 4C corner-block row idx32[p, k, t] of the table
                g4 = gpool.tile([128, NT, 4 * C], BF16, tag="g4", name="g4")
                for t in range(NT):
                    nc.gpsimd.indirect_dma_start(
                        out=g4[:, t, :], out_offset=None,
                        in_=xcl4_d.ap()[:, :],
                        in_offset=bass.IndirectOffsetOnAxis(
                            ap=idx32[:, k, t:t + 1], axis=0))
                gv = {"00": lambda t: g4[:, t, 0:C],
                      "01": lambda t: g4[:, t, C:2 * C],
                      "10": lambda t: g4[:, t, 2 * C:3 * C],
                      "11": lambda t: g4[:, t, 3 * C:4 * C]}
            else:
                g0 = gpool.tile([128, NT, 2 * C], BF16, tag="g0", name="g0")
                g1 = gpool.tile([128, NT, 2 * C], BF16, tag="g1", name="g1")
                # NI-idx gathers per (k, y-corner); position i = pixel i,
                # its index at iw[i%16 (+16a), 128*ks + i//16]
                NI = 512
                NQ = NPIX // NI
                for q in range(NQ):
                    nc.gpsimd.dma_gather(
                        out_ap=g0[:, (NI // 128) * q:(NI // 128) * (q + 1), :],
                        in_ap=win0,
                        idxs_ap=iw[:, 128 * k + (NI // 16) * q:
                                   128 * k + (NI // 16) * (q + 1)],
                        num_idxs=NI, num_idxs_reg=NI,
                        elem_size=2 * C, elem_step=C)
                    nc.gpsimd.dma_gather(
                        out_ap=g1[:, (NI // 128) * q:(NI // 128) * (q + 1), :],
                        in_ap=win0,
                        idxs_ap=iw[:, 128 * (9 + k) + (NI // 16) * q:
                                   128 * (9 + k) + (NI // 16) * (q + 1)],
                        num_idxs=NI, num_idxs_reg=NI,
                        elem_size=2 * C, elem_step=C)
                gv = {"00": lambda t: g0[:, t, 0:C],
                      "01": lambda t: g0[:, t, C:2 * C],
                      "10": lambda t: g1[:, t, 0:C],
                      "11": lambda t: g1[:, t, C:2 * C]}

            if STAGE < 4:
                continue
            val = vpool.tile([128, 2, NPIX], BF16, tag="val", name="val")
            vals.append(val)
            for half in range(4):      # 4 pixel-quads of 4 tiles each
                tp = [tp_ps.tile([128, 512], BF16, tag="tp", name="tp") for _ in range(2)]
                for t in range(4 * half, 4 * half + 4):
                    col = t * 9 + k
                    # val_t = c00*g0lo + c10*g1lo + c01*g0hi + c11*g1hi
                    # via 1 scalar-engine mult + 3 fused DVE mult-adds
                    mb = apool.tile([128, C], BF16, tag="mb", name="mb")
                    nc.scalar.activation(mb[:, :], gv["00"](t),
                                         AF.Copy, scale=c00[:, col:col + 1])
                    m1 = apool.tile([128, C], BF16, tag="m1", name="m1")
                    nc.vector.scalar_tensor_tensor(
                        m1[:, :], gv["10"](t), cb[2][:, col:col + 1],
                        mb[:, :], OP.mult, OP.add)
                    m2 = apool.tile([128, C], BF16, tag="m2", name="m2")
                    nc.vector.scalar_tensor_tensor(
                        m2[:, :], gv["01"](t), cb[1][:, col:col + 1],
                        m1[:, :], OP.mult, OP.add)
                    vt = apool.tile([128, C], BF16, tag="vt", name="vt")
                    nc.vector.scalar_tensor_tensor(
                        vt[:, :], gv["11"](t), cb[3][:, col:col + 1],
                        m2[:, :], OP.mult, OP.add)
                    # PE transpose pixel-major val tile -> C-major (PSUM)
                    sl = slice((t % 4) * 128, (t % 4) * 128 + 128)
                    for ch in range(2):
                        nc.tensor.matmul(tp[ch][:, sl],
                                         vt[:, ch * 128:(ch + 1) * 128],
                                         identb[:, :], is_transpose=True,
                                         start=True, stop=True)
                for ch in range(2):
                    nc.scalar.copy(val[:, ch, half * 512:(half + 1) * 512],
                                   tp[ch][:, :])
            if STAGE >= 5 and k >= 1:
                emit_gemm_k(gps0, k - 1, ot=0)

        if STAGE < 5:
            _finish_stub(nc, tc, cpool, y_d)
            return
        emit_gemm_k(gps0, 8, ot=0)
        out0 = opool.tile([128, NPIX], F32, tag="outs", name="outs")
        for nb in range(4):
            nc.scalar.copy(out0[:, nb * 512:(nb + 1) * 512], gps0[nb][:, :])
        nc.sync.dma_start(y_d.ap()[0], out0[:, :])

        gps1 = [gemm_ps.tile([128, 512], F32, tag="gps", name="gps") for _ in range(4)]
        for k in range(9):
            emit_gemm_k(gps1, k, ot=1)
        out1 = opool.tile([128, NPIX], F32, tag="outs", name="outs")
        for nb in range(4):
            nc.scalar.copy(out1[:, nb * 512:(nb + 1) * 512], gps1[nb][:, :])
        nc.sync.dma_start(y_d.ap()[1], out1[:, :])


# ---------------------------------------------------------------------------
# Host side
# ---------------------------------------------------------------------------

def make_core_inputs(x, w_off, b_off, w_conv, core):
    b, s = core // 2, core % 2
    h0 = 32 * s
    xp = np.zeros((C, Hp, Wp), np.float32)
    xp[:, PADR:PADR + H, PADR:PADR + W] = x[b]
    xcl = np.ascontiguousarray(xp.transpose(1, 2, 0)).reshape(-1).astype(BbArr)
    # 4C corner-block table: token (y,x) = [x[y,x], x[y,x+1], x[y+1,x],
    # x[y+1,x+1]] channels-last, for the hw-DGE indirect gather
    xpl = xp.transpose(1, 2, 0)                      # [Hp, Wp, C]
    x4 = np.zeros((Hp, Wp, 4, C), np.float32)
    x4[:-1, :-1, 0] = xpl[:-1, :-1]
    x4[:-1, :-1, 1] = xpl[:-1, 1:]
    x4[:-1, :-1, 2] = xpl[1:, :-1]
    x4[:-1, :-1, 3] = xpl[1:, 1:]
    xcl4 = x4.reshape(Hp * Wp, 4 * C).astype(BbArr)
    xcm = np.ascontiguousarray(
        xp.reshape(C, Hp * Wp)[:, h0 * Wp: h0 * Wp + XCM_COLS]
    ).astype(BbArr).reshape(2, 128, XCM_COLS)

    # w_off [27, C, 3, 3] -> [2, 128, 9*27]: [ct, c, tap*27+oc]
    wof = w_off.transpose(1, 2, 3, 0).reshape(2, 128, 9, 27).reshape(2, 128, 243)
    wof = np.ascontiguousarray(wof).astype(BbArr)
    # w_conv [O, C, 3, 3] -> [2(ct), 128(c), (ot*9+k)*128+o]
    wc = w_conv.reshape(2, 128, C, 9).transpose(2, 0, 3, 1)   # [c, ot, k, o128]
    wc = np.ascontiguousarray(wc.reshape(2, 128, 2 * 9 * 128)).astype(BbArr)

    r = np.arange(128)[:, None, None]
    t = np.arange(NT)[None, :, None]
    kk = np.arange(9)[None, None, :]
    gyk = (h0 + 2 * t + r // 64 - 1 + kk // 3).astype(np.float32)
    gyk = np.broadcast_to(gyk, (128, NT, 9)).reshape(128, 144).copy()
    gxk = (r % 64 - 1 + kk % 3).astype(np.float32)
    gxk = np.broadcast_to(gxk, (128, NT, 9)).reshape(128, 144).copy()

    # selection matrices for the 128->16 index partition wrap:
    # S_j[p, m] = 1 iff p == 16j + m%16  (j = r//16 column-block)
    srep = np.zeros((128, 8, 128), np.float32)
    p = np.arange(128)
    m = np.arange(128)
    for j in range(8):
        srep[:, j, :] = (p[:, None] == 16 * j + (m[None, :] % 16))
    srep = srep.reshape(128, 1024)

    return {
        "xcl": xcl,
        "xcl4": xcl4,
        "xcm": xcm,
        "woff": wof,
        "wcv": wc,
        "gyk": gyk,
        "gxk": gxk,
        "boff": b_off.reshape(27, 1).astype(np.float32),
        "identb": np.eye(128, dtype=np.float32).astype(BbArr),
        "identf": np.eye(32, dtype=np.float32),
        "srep": srep,
    }


BbArr = ml_dtypes.bfloat16

_NC = None


def kernel(x, w_off, b_off, w_conv):
    global _NC
    x = np.asarray(x, np.float32)
    w_off = np.asarray(w_off, np.float32)
    b_off = np.asarray(b_off, np.float32)
    w_conv = np.asarray(w_conv, np.float32)
    if _NC is None:
        _NC = build_program()
    in_maps = [make_core_inputs(x, w_off, b_off, w_conv, c)
               for c in range(N_CORES)]
    res = run_bass_kernel_spmd(_NC, in_maps, core_ids=list(range(N_CORES)))
    out = np.empty((B, O, H, W), np.float32)
    for c in range(N_CORES):
        b, s = c // 2, c % 2
        out[b, :, 32 * s:32 * s + 32, :] = res.results[c]["y"].reshape(O, 32, W)
    return out

